# revision 24
# baseline (speedup 1.0000x reference)
"""Griffin-Lim phase reconstruction on Trainium2 (Bass/Tile) — v2.

Structure (per core; core c handles batch element c%4, cores 4-7 duplicate):
  * Crop: only the first TC=64 of 1000 STFT frames can affect the returned
    1000 samples (validated numerically: rel err ~6e-5; see mock_numerics.py).
  * Phase carried as (mag*cos, mag*sin); no trig in the loop.
  * ISTFT: 4 K=128 fp32 matmuls accumulate BOTH 128-sample halves into one
    PSUM tile P[128, 3+TC] with the second half at a +4 column offset, so the
    8-way overlap-add collapses to a 2-level tree:
      u[64,1+TC] (PSUM)  = P[0:64] + P[64:128] shifted 2   (1 DVE op)
      v[32,TC]   (SBUF)  = u[0:32] + u[32:64] shifted 1    (1 DVE op)
  * The 1/win_sq normalization is fused into the STFT frame gather: the
    gather is 4 tensor_mul ops ga[32j:32j+32,:] = v[:, j:...] * invw[:, j:...]
    (gb would be ga shifted 4 columns, so only ga[128, TS+4] is materialized
    and the STFT matmuls read ga[:,0:TS] and ga[:,4:4+TS]).
  * STFT: 4 K=128 fp32 matmuls -> t2r (Re f=0..127), t2i (row0 = Re Nyquist,
    rows 1..127 = Im f=1..127).
  * Norm z/|z| without materializing phase: w2 = t2r^2+t2i^2, |z| = Sqrt
    (ACT, the only ACT op), 1/|z| via reciprocal_approx_fast (single DVE op),
    sa/sb rows 1..127 = mag * t2 * inv; rows 0 (DC/Nyquist, where Im==0) are
    sign-patched with mag*sign(Re), scheduled on DVE during the ACT Sqrt.
"""

import numpy as np
from contextlib import ExitStack

import concourse.bass as bass
import concourse.tile as tile
from concourse import bacc, mybir
from concourse import bass_utils

F32 = mybir.dt.float32
I32 = mybir.dt.int32
AF = mybir.ActivationFunctionType
OP = mybir.AluOpType

TC = 64           # cropped frame count (of 1000)
TS = TC - 7       # stft / phase-update frame count
PAD = 7
LP = 3            # left zero-pad of the fused ISTFT psum tile
N_ITER = 32
N_FFT = 256
NF = 129
HOP = 32
N_CORES = 8
B = 4


def _consts():
    n = np.arange(N_FFT, dtype=np.float64)
    win = 0.5 - 0.5 * np.cos(2.0 * np.pi * n / N_FFT)
    k = np.arange(128, dtype=np.float64)[:, None]
    ang = 2.0 * np.pi * k * n[None, :] / N_FFT
    ck = np.where(k == 0, 1.0, 2.0) / N_FFT
    a_r = (ck * np.cos(ang) * win[None, :]).astype(np.float32)       # (128,256)
    a_i = (-2.0 / N_FFT * np.sin(ang) * win[None, :]).astype(np.float32)
    a_i[0] = (np.cos(np.pi * n) / N_FFT * win).astype(np.float32)    # Nyquist row

    f = np.arange(128, dtype=np.float64)[None, :]
    ang2 = 2.0 * np.pi * f * n[:, None] / N_FFT                      # (256,128)
    bc = (win[:, None] * np.cos(ang2)).astype(np.float32)
    bi = (-win[:, None] * np.sin(ang2)).astype(np.float32)
    bi[:, 0] = (win * np.cos(np.pi * n)).astype(np.float32)

    L = TC * HOP
    wsq = np.zeros((TC + 8) * HOP + N_FFT, dtype=np.float64)
    w2 = win ** 2
    for t in range(TC + 8):
        s = t * HOP
        wsq[s:s + N_FFT] += w2
    wsq = np.maximum(wsq[:L], 1e-8)
    invwsq = (1.0 / wsq).astype(np.float32).reshape(TC, HOP).T.copy()  # (32, TC)
    return a_r, a_i, bc.copy(), bi.copy(), invwsq


def _emit(tc_ctx, aps, rep=1):
    tc = tc_ctx
    nc = tc.nc
    with ExitStack() as ctx:
        consts = ctx.enter_context(tc.tile_pool(name="consts", bufs=1))
        state = ctx.enter_context(tc.tile_pool(name="state", bufs=1))
        work = ctx.enter_context(tc.tile_pool(name="work", bufs=2))
        psum = ctx.enter_context(tc.tile_pool(name="psum", bufs=1, space="PSUM"))

        a_r = consts.tile([128, 256], F32)
        a_i = consts.tile([128, 256], F32)
        bca = consts.tile([128, 128], F32)
        bcb = consts.tile([128, 128], F32)
        bia = consts.tile([128, 128], F32)
        bib = consts.tile([128, 128], F32)
        invw = consts.tile([32, TC], F32)
        maga = consts.tile([128, TS], F32)
        magn = consts.tile([1, TS], F32)
        binyq = consts.tile([128, 2], F32)   # cols: [bia_nyq | bib_nyq]
        sa = state.tile([128, TC + 2 * PAD], F32)
        sb = state.tile([128, TC + 2 * PAD], F32)

        for t, name in [(a_r, "a_r"), (a_i, "a_i"), (bca, "bca"), (bcb, "bcb"),
                        (bia, "bia"), (bib, "bib"),
                        (invw, "invw"), (maga, "maga"), (magn, "magn"),
                        (binyq, "binyq")]:
            nc.sync.dma_start(out=t, in_=aps[name])

        if rep > 1:
            from concourse.engine_type import EngineType
            loop = tc.For_i(0, rep, 1, hint_engines=(
                EngineType.PE, EngineType.DVE, EngineType.Activation,
                EngineType.SP))
        else:
            loop = None
        if loop is not None:
            loop.__enter__()
        nc.sync.dma_start(out=sa, in_=aps["sa0"])
        nc.sync.dma_start(out=sb, in_=aps["sb0"])

        for it in range(N_ITER):
            last = it == N_ITER - 1
            # ---- fused ISTFT (+pair overlap-add): P[32g+i, LP+c] =
            #      p1[32g+i, c] + p2[32g+i, c-4], groups g=0..3 ----
            P = psum.tile([128, LP + TC], F32, tag="P")
            # sa-fed matmuls first: they only wait on the sa write, while the
            # sb write + Nyquist patch finish in parallel.
            nc.tensor.matmul(P[:, 0:LP + TC], a_r[:, 0:128],
                             sa[:, PAD - LP:PAD + TC], start=True, stop=False)
            nc.tensor.matmul(P[:, LP + 4:LP + TC], a_r[:, 128:256],
                             sa[:, PAD:PAD + TC - 4], start=False, stop=False)
            nc.tensor.matmul(P[:, 0:LP + TC], a_i[:, 0:128],
                             sb[:, PAD - LP:PAD + TC], start=False, stop=False)
            nc.tensor.matmul(P[:, LP + 4:LP + TC], a_i[:, 128:256],
                             sb[:, PAD:PAD + TC - 4], start=False, stop=True)
            # ---- overlap-add: walrus allows at most ONE PSUM operand per
            #      elementwise op, so fold the 4 partition groups with an
            #      in-place chain (copy + 3 shifted accumulates) ----
            v = work.tile([32, TC], F32, tag="v")
            nc.vector.tensor_copy(v, P[0:32, LP:LP + TC])
            nc.vector.tensor_add(v, v, P[32:64, LP - 1:LP + TC - 1])
            nc.vector.tensor_add(v, v, P[64:96, LP - 2:LP + TC - 2])
            nc.vector.tensor_add(v, v, P[96:128, LP - 3:LP + TC - 3])

            if last:
                wav = work.tile([32, 32], F32, tag="wav")
                nc.vector.tensor_mul(wav, v[:, 0:32], invw[:, 0:32])
                nc.sync.dma_start(out=aps["out"], in_=wav)
                break

            # ---- frame gather with fused 1/win_sq scaling ----
            ga = work.tile([128, TS + 4], F32, tag="ga")
            for j in range(4):
                nc.vector.tensor_mul(ga[32 * j:32 * j + 32, :],
                                     v[:, j:j + TS + 4], invw[:, j:j + TS + 4])
            # ---- STFT: 4 K=128 matmuls (gb == ga shifted 4 columns).
            #      bia/bib have column 0 (Nyquist) zeroed, so t2i row 0 ==
            #      Im(DC) == 0 and the generic norm is exact for row 0 of sa;
            #      Re(Nyquist) is produced separately by two M=1 matmuls. ----
            t2r = psum.tile([128, TS], F32, tag="t2r")
            t2i = psum.tile([128, TS], F32, tag="t2i")
            t2n = psum.tile([1, TS], F32, tag="t2n")
            nc.tensor.matmul(t2r, bca, ga[:, 0:TS], start=True, stop=False)
            nc.tensor.matmul(t2r, bcb, ga[:, 4:4 + TS], start=False, stop=True)
            nc.tensor.matmul(t2i, bia, ga[:, 0:TS], start=True, stop=False)
            nc.tensor.matmul(t2i, bib, ga[:, 4:4 + TS], start=False, stop=True)
            nc.tensor.matmul(t2n, binyq[:, 0:1], ga[:, 0:TS],
                             start=True, stop=False)
            nc.tensor.matmul(t2n, binyq[:, 1:2], ga[:, 4:4 + TS],
                             start=False, stop=True)

            # ---- phase projection: s' = mag * t2 / |t2|, entirely on DVE
            #      (cross-engine sync costs ~0.5us on HW, so no ACT round
            #      trips: 1/|t2| via bit-trick seed + 2 Newton steps) ----
            cr = work.tile([128, TS], F32, tag="cr")
            ci = work.tile([128, TS], F32, tag="ci")
            nc.vector.tensor_copy(cr, t2r)
            nc.vector.tensor_copy(ci, t2i)
            geB = work.tile([1, TS], F32, tag="geB")
            nc.vector.tensor_scalar(geB, t2n[0:1, :], -1e-6, 2.0,
                                    OP.is_ge, OP.mult)
            u2 = work.tile([128, TS], F32, tag="u2")
            v2 = work.tile([128, TS], F32, tag="v2")
            w2 = work.tile([128, TS], F32, tag="w2")
            nc.vector.tensor_mul(u2, cr, cr)
            nc.vector.tensor_mul(v2, ci, ci)
            nc.vector.scalar_tensor_tensor(w2, u2, 1e-12, v2, OP.add, OP.add)
            # y0 = bits(0x5f3759df - (bits(w2) >> 1)); note -x == ~x + 1
            y0 = work.tile([128, TS], F32, tag="y0")
            sh = work.tile([128, TS], F32, tag="sh")
            nc.vector.tensor_scalar(sh[:, :].bitcast(I32), w2[:, :].bitcast(I32),
                                    1, -1, OP.logical_shift_right, OP.bitwise_xor)
            nc.vector.tensor_scalar(y0[:, :].bitcast(I32), sh[:, :].bitcast(I32),
                                    0x5F3759E0, None, OP.add)
            yy = y0
            for step in range(2):
                ysq = work.tile([128, TS], F32, tag=f"ysq{step}")
                tq = work.tile([128, TS], F32, tag=f"tq{step}")
                st = work.tile([128, TS], F32, tag=f"st{step}")
                yn = work.tile([128, TS], F32, tag=f"yn{step}")
                nc.vector.tensor_mul(ysq, yy, yy)
                nc.vector.tensor_mul(tq, w2, ysq)
                nc.vector.tensor_scalar(st, tq, -0.5, 1.5, OP.mult, OP.add)
                nc.vector.tensor_mul(yn, yy, st)
                yy = yn
            pm = work.tile([128, TS], F32, tag="pm")
            nc.vector.tensor_mul(pm, maga, yy)
            nc.vector.tensor_mul(sa[:, PAD:PAD + TS], t2r, pm)
            nc.vector.tensor_mul(sb[:, PAD:PAD + TS], t2i, pm)
            # Nyquist row of sb: mag*sign(Re+eps)
            nc.vector.scalar_tensor_tensor(sb[0:1, PAD:PAD + TS], geB, 1.0,
                                           magn, OP.subtract, OP.mult)
        if loop is not None:
            loop.__exit__(None, None, None)


_CACHED = None


def _build(rep=1):
    global _CACHED
    if rep == 1 and _CACHED is not None:
        return _CACHED
    nc = bacc.Bacc("TRN2", target_bir_lowering=False, debug=False,
                   num_devices=N_CORES)
    shapes = {
        "a_r": (128, 256), "a_i": (128, 256), "bca": (128, 128),
        "bcb": (128, 128), "bia": (128, 128), "bib": (128, 128),
        "invw": (32, TC), "maga": (128, TS),
        "magn": (1, TS), "binyq": (128, 2),
        "sa0": (128, TC + 2 * PAD), "sb0": (128, TC + 2 * PAD),
    }
    aps = {name: nc.dram_tensor(name, shape, F32, kind="ExternalInput").ap()
           for name, shape in shapes.items()}
    aps["out"] = nc.dram_tensor("out", (32, 32), F32, kind="ExternalOutput").ap()
    with tile.TileContext(nc) as t:
        _emit(t, aps, rep=rep)
    nc.compile()
    if rep == 1:
        _CACHED = nc
    return nc


def _host_inputs(mag_b, ph_b):
    """Per-batch host prep: crop, initial cos/sin spec chunks, padding."""
    a_r, a_i, bc, bi, invwsq = _consts()
    mag = np.ascontiguousarray(mag_b[:, :TC]).astype(np.float32)
    ph = np.ascontiguousarray(ph_b[:, :TC]).astype(np.float32)
    sa0 = np.zeros((128, TC + 2 * PAD), np.float32)
    sb0 = np.zeros((128, TC + 2 * PAD), np.float32)
    sa0[:, PAD:PAD + TC] = mag[0:128] * np.cos(ph[0:128])
    sb0[0, PAD:PAD + TC] = mag[128] * np.cos(ph[128])
    sb0[1:, PAD:PAD + TC] = mag[1:128] * np.sin(ph[1:128])
    bia = np.ascontiguousarray(bi[0:128])
    bib = np.ascontiguousarray(bi[128:256])
    binyq = np.stack([bia[:, 0], bib[:, 0]], axis=1).copy()  # (128, 2)
    bia[:, 0] = 0.0   # t2i row 0 becomes Im(DC) == 0 by construction
    bib[:, 0] = 0.0
    return {
        "a_r": a_r, "a_i": a_i,
        "bca": np.ascontiguousarray(bc[0:128]), "bcb": np.ascontiguousarray(bc[128:256]),
        "bia": bia, "bib": bib, "binyq": binyq,
        "invw": invwsq,
        "maga": np.ascontiguousarray(mag[0:128, :TS]),
        "magn": np.ascontiguousarray(mag[128:129, :TS]),
        "sa0": sa0, "sb0": sb0,
    }


def kernel(mag_spec, phase):
    mag_spec = np.asarray(mag_spec, dtype=np.float32)
    phase = np.asarray(phase, dtype=np.float32)
    nc = _build()
    in_maps = [_host_inputs(mag_spec[c % B], phase[c % B]) for c in range(N_CORES)]
    res = bass_utils.run_bass_kernel_spmd(nc, in_maps, core_ids=list(range(N_CORES)))
    out = np.zeros((B, 1000), np.float32)
    for b in range(B):
        blk = res.results[b]["out"]              # (32, 32): [i, m] = wav[32m+i]
        out[b] = blk.T.reshape(-1)[15:1015]
    return out


# revision 37
# speedup vs baseline: 1.6620x; 1.6620x over previous
"""Griffin-Lim phase reconstruction on Trainium2 (Bass/Tile) — v2.

Structure (per core; core c handles batch element c%4, cores 4-7 duplicate):
  * Crop: only the first TC=64 of 1000 STFT frames can affect the returned
    1000 samples (validated numerically: rel err ~6e-5; see mock_numerics.py).
  * Phase carried as (mag*cos, mag*sin); no trig in the loop.
  * ISTFT: 4 K=128 fp32 matmuls accumulate BOTH 128-sample halves into one
    PSUM tile P[128, 3+TC] with the second half at a +4 column offset, so the
    8-way overlap-add collapses to a 2-level tree:
      u[64,1+TC] (PSUM)  = P[0:64] + P[64:128] shifted 2   (1 DVE op)
      v[32,TC]   (SBUF)  = u[0:32] + u[32:64] shifted 1    (1 DVE op)
  * The 1/win_sq normalization is fused into the STFT frame gather: the
    gather is 4 tensor_mul ops ga[32j:32j+32,:] = v[:, j:...] * invw[:, j:...]
    (gb would be ga shifted 4 columns, so only ga[128, TS+4] is materialized
    and the STFT matmuls read ga[:,0:TS] and ga[:,4:4+TS]).
  * STFT: 4 K=128 fp32 matmuls -> t2r (Re f=0..127), t2i (row0 = Re Nyquist,
    rows 1..127 = Im f=1..127).
  * Norm z/|z| without materializing phase: w2 = t2r^2+t2i^2, |z| = Sqrt
    (ACT, the only ACT op), 1/|z| via reciprocal_approx_fast (single DVE op),
    sa/sb rows 1..127 = mag * t2 * inv; rows 0 (DC/Nyquist, where Im==0) are
    sign-patched with mag*sign(Re), scheduled on DVE during the ACT Sqrt.
"""

import numpy as np
from contextlib import ExitStack

import concourse.bass as bass
import concourse.tile as tile
from concourse import bacc, mybir
from concourse import bass_utils

F32 = mybir.dt.float32
I32 = mybir.dt.int32
AF = mybir.ActivationFunctionType
OP = mybir.AluOpType

TC = 64           # cropped frame count (of 1000)
TS = TC - 7       # stft / phase-update frame count
PAD = 7
EPSNY = 2.0 ** -20   # Nyquist basis scale (keeps sign info, kills norm term)
N_ITER = 32
N_FFT = 256
NF = 129
HOP = 32
N_CORES = 8
B = 4


def _consts():
    n = np.arange(N_FFT, dtype=np.float64)
    win = 0.5 - 0.5 * np.cos(2.0 * np.pi * n / N_FFT)
    k = np.arange(128, dtype=np.float64)[:, None]
    ang = 2.0 * np.pi * k * n[None, :] / N_FFT
    ck = np.where(k == 0, 1.0, 2.0) / N_FFT
    a_r = (ck * np.cos(ang) * win[None, :]).astype(np.float32)       # (128,256)
    a_i = (-2.0 / N_FFT * np.sin(ang) * win[None, :]).astype(np.float32)
    a_i[0] = (np.cos(np.pi * n) / N_FFT * win).astype(np.float32)    # Nyquist row

    f = np.arange(128, dtype=np.float64)[None, :]
    ang2 = 2.0 * np.pi * f * n[:, None] / N_FFT                      # (256,128)
    bc = (win[:, None] * np.cos(ang2)).astype(np.float32)
    bi = (-win[:, None] * np.sin(ang2)).astype(np.float32)
    bi[:, 0] = (win * np.cos(np.pi * n)).astype(np.float32)

    L = TC * HOP
    wsq = np.zeros((TC + 8) * HOP + N_FFT, dtype=np.float64)
    w2 = win ** 2
    for t in range(TC + 8):
        s = t * HOP
        wsq[s:s + N_FFT] += w2
    wsq = np.maximum(wsq[:L], 1e-8)
    invwsq = (1.0 / wsq).astype(np.float32).reshape(TC, HOP).T.copy()  # (32, TC)
    return a_r, a_i, bc.copy(), bi.copy(), invwsq


def _emit(tc_ctx, aps, rep=1):
    tc = tc_ctx
    nc = tc.nc
    with ExitStack() as ctx:
        consts = ctx.enter_context(tc.tile_pool(name="consts", bufs=1))
        state = ctx.enter_context(tc.tile_pool(name="state", bufs=1))
        work = ctx.enter_context(tc.tile_pool(name="work", bufs=2))
        psum = ctx.enter_context(tc.tile_pool(name="psum", bufs=1, space="PSUM"))

        a_r = consts.tile([128, 256], F32)
        a_i = consts.tile([128, 256], F32)
        bca = consts.tile([128, 128], F32)
        bcb = consts.tile([128, 128], F32)
        bia = consts.tile([128, 128], F32)
        bib = consts.tile([128, 128], F32)
        invw = consts.tile([32, TC], F32)
        maga = consts.tile([128, TS], F32)
        magn = consts.tile([1, TS], F32)
        W = TC + 2 * PAD
        sab = state.tile([128, 2 * W], F32)   # [sa | sb], one DMA per rep
        sa = sab[:, 0:W]
        sb = sab[:, W:2 * W]

        for t, name in [(a_r, "a_r"), (a_i, "a_i"), (bca, "bca"), (bcb, "bcb"),
                        (bia, "bia"), (bib, "bib"),
                        (invw, "invw"), (maga, "maga"), (magn, "magn")]:
            nc.sync.dma_start(out=t, in_=aps[name])

        if rep > 1:
            from concourse.engine_type import EngineType
            loop = tc.For_i(0, rep, 1, hint_engines=(
                EngineType.PE, EngineType.DVE, EngineType.Activation,
                EngineType.SP))
        else:
            loop = None
        if loop is not None:
            loop.__enter__()
        nc.sync.dma_start(out=sab, in_=aps["sab0"])

        for it in range(N_ITER):
            last = it == N_ITER - 1
            # ---- ISTFT: two PSUM banks (accumulation chains on a single
            #      bank serialize at ~1.2us/matmul on HW; alternating banks
            #      pipeline). sa-fed matmuls first: they only wait on the sa
            #      write, while the sb write + Nyquist patch finish. ----
            p1 = psum.tile([128, TC], F32, tag="p1")   # samples n=0..127
            p2 = psum.tile([128, TC], F32, tag="p2")   # samples n=128..255
            nc.tensor.matmul(p1, a_r[:, 0:128], sa[:, PAD:PAD + TC],
                             start=True, stop=False)
            nc.tensor.matmul(p2, a_r[:, 128:256], sa[:, PAD:PAD + TC],
                             start=True, stop=False)
            nc.tensor.matmul(p1, a_i[:, 0:128], sb[:, PAD:PAD + TC],
                             start=False, stop=True)
            nc.tensor.matmul(p2, a_i[:, 128:256], sb[:, PAD:PAD + TC],
                             start=False, stop=True)
            # ---- overlap-add: in-place DVE chain (~63ns/op on HW); at most
            #      one PSUM operand per elementwise op ----
            v = work.tile([32, TC], F32, tag="v")
            nc.vector.tensor_copy(v, p1[0:32, :])
            for j in range(1, 4):
                nc.vector.tensor_add(v[:, j:TC], v[:, j:TC],
                                     p1[32 * j:32 * j + 32, 0:TC - j])
            for j in range(4, 8):
                nc.vector.tensor_add(v[:, j:TC], v[:, j:TC],
                                     p2[32 * (j - 4):32 * (j - 4) + 32, 0:TC - j])

            if last:
                wav = work.tile([32, 32], F32, tag="wav")
                nc.vector.tensor_mul(wav, v[:, 0:32], invw[:, 0:32])
                nc.sync.dma_start(out=aps["out"], in_=wav)
                break

            # ---- frame gather with fused 1/win_sq scaling ----
            ga = work.tile([128, TS + 4], F32, tag="ga")
            for j in range(4):
                nc.vector.tensor_mul(ga[32 * j:32 * j + 32, :],
                                     v[:, j:j + TS + 4], invw[:, j:j + TS + 4])
            # ---- STFT: 4 K=128 matmuls on two banks (gb == ga shifted 4
            #      columns). bia/bib column 0 (Nyquist basis) is scaled by
            #      EPSNY, so t2i row 0 ~ 0 and the generic norm stays exact
            #      for row 0 of sa; the Nyquist sign survives in t2i[0]. ----
            t2r = psum.tile([128, TS], F32, tag="t2r")
            t2i = psum.tile([128, TS], F32, tag="t2i")
            nc.tensor.matmul(t2r, bca, ga[:, 0:TS], start=True, stop=False)
            nc.tensor.matmul(t2i, bia, ga[:, 0:TS], start=True, stop=False)
            nc.tensor.matmul(t2r, bcb, ga[:, 4:4 + TS], start=False, stop=True)
            nc.tensor.matmul(t2i, bib, ga[:, 4:4 + TS], start=False, stop=True)

            # ---- phase projection: s' = mag * t2 / |t2| ----
            # DVE squares need SBUF copies (<=1 PSUM operand per op); the
            # copies also serve the final sa/sb muls.
            cr = work.tile([128, TS], F32, tag="cr")
            ci = work.tile([128, TS], F32, tag="ci")
            nc.vector.tensor_copy(cr, t2r)
            nc.vector.tensor_copy(ci, t2i)
            geB = work.tile([1, TS], F32, tag="geB")
            nc.vector.tensor_scalar(geB, ci[0:1, :], -1e-6 * EPSNY, 2.0,
                                    OP.is_ge, OP.mult)
            u2 = work.tile([128, TS], F32, tag="u2")
            v2 = work.tile([128, TS], F32, tag="v2")
            w2 = work.tile([128, TS], F32, tag="w2")
            nc.vector.tensor_mul(u2, cr, cr)
            nc.vector.tensor_mul(v2, ci, ci)
            nc.vector.scalar_tensor_tensor(w2, u2, 1e-12, v2, OP.add, OP.add)
            hyp = work.tile([128, TS], F32, tag="hyp")
            nc.scalar.activation(hyp, w2, AF.Sqrt)
            inv = work.tile([128, TS], F32, tag="inv")
            nc.vector.reciprocal_approx_fast(inv, hyp)
            pm = work.tile([128, TS], F32, tag="pm")
            nc.vector.tensor_mul(pm, maga, inv)
            nc.vector.tensor_mul(sa[:, PAD:PAD + TS], cr, pm)
            nc.vector.tensor_mul(sb[:, PAD:PAD + TS], ci, pm)
            nc.vector.scalar_tensor_tensor(sb[0:1, PAD:PAD + TS], geB, 1.0,
                                           magn, OP.subtract, OP.mult)
        if loop is not None:
            loop.__exit__(None, None, None)


_CACHED = None


def _build(rep=1):
    global _CACHED
    if rep == 1 and _CACHED is not None:
        return _CACHED
    nc = bacc.Bacc("TRN2", target_bir_lowering=False, debug=False,
                   num_devices=N_CORES)
    shapes = {
        "a_r": (128, 256), "a_i": (128, 256), "bca": (128, 128),
        "bcb": (128, 128), "bia": (128, 128), "bib": (128, 128),
        "invw": (32, TC), "maga": (128, TS),
        "magn": (1, TS),
        "sab0": (128, 2 * (TC + 2 * PAD)),
    }
    aps = {name: nc.dram_tensor(name, shape, F32, kind="ExternalInput").ap()
           for name, shape in shapes.items()}
    aps["out"] = nc.dram_tensor("out", (32, 32), F32, kind="ExternalOutput").ap()
    with tile.TileContext(nc) as t:
        _emit(t, aps, rep=rep)
    nc.compile()
    if rep == 1:
        _CACHED = nc
    return nc


def _host_inputs(mag_b, ph_b):
    """Per-batch host prep: crop, initial cos/sin spec chunks, padding."""
    a_r, a_i, bc, bi, invwsq = _consts()
    mag = np.ascontiguousarray(mag_b[:, :TC]).astype(np.float32)
    ph = np.ascontiguousarray(ph_b[:, :TC]).astype(np.float32)
    W = TC + 2 * PAD
    sab0 = np.zeros((128, 2 * W), np.float32)
    sa0 = sab0[:, 0:W]
    sb0 = sab0[:, W:2 * W]
    sa0[:, PAD:PAD + TC] = mag[0:128] * np.cos(ph[0:128])
    sb0[0, PAD:PAD + TC] = mag[128] * np.cos(ph[128])
    sb0[1:, PAD:PAD + TC] = mag[1:128] * np.sin(ph[1:128])
    bia = np.ascontiguousarray(bi[0:128])
    bib = np.ascontiguousarray(bi[128:256])
    bia[:, 0] *= EPSNY   # t2i row 0 ~ 0 in the norm, keeps the Nyquist sign
    bib[:, 0] *= EPSNY
    return {
        "a_r": a_r, "a_i": a_i,
        "bca": np.ascontiguousarray(bc[0:128]), "bcb": np.ascontiguousarray(bc[128:256]),
        "bia": bia, "bib": bib,
        "invw": invwsq,
        "maga": np.ascontiguousarray(mag[0:128, :TS]),
        "magn": np.ascontiguousarray(mag[128:129, :TS]),
        "sab0": sab0,
    }


def kernel(mag_spec, phase):
    mag_spec = np.asarray(mag_spec, dtype=np.float32)
    phase = np.asarray(phase, dtype=np.float32)
    nc = _build()
    in_maps = [_host_inputs(mag_spec[c % B], phase[c % B]) for c in range(N_CORES)]
    res = bass_utils.run_bass_kernel_spmd(nc, in_maps, core_ids=list(range(N_CORES)))
    out = np.zeros((B, 1000), np.float32)
    for b in range(B):
        blk = res.results[b]["out"]              # (32, 32): [i, m] = wav[32m+i]
        out[b] = blk.T.reshape(-1)[15:1015]
    return out


# revision 40
# speedup vs baseline: 1.7736x; 1.0671x over previous
"""Griffin-Lim phase reconstruction on Trainium2 (Bass/Tile) — v2.

Structure (per core; core c handles batch element c%4, cores 4-7 duplicate):
  * Crop: only the first TC=64 of 1000 STFT frames can affect the returned
    1000 samples (validated numerically: rel err ~6e-5; see mock_numerics.py).
  * Phase carried as (mag*cos, mag*sin); no trig in the loop.
  * ISTFT: 4 K=128 fp32 matmuls accumulate BOTH 128-sample halves into one
    PSUM tile P[128, 3+TC] with the second half at a +4 column offset, so the
    8-way overlap-add collapses to a 2-level tree:
      u[64,1+TC] (PSUM)  = P[0:64] + P[64:128] shifted 2   (1 DVE op)
      v[32,TC]   (SBUF)  = u[0:32] + u[32:64] shifted 1    (1 DVE op)
  * The 1/win_sq normalization is fused into the STFT frame gather: the
    gather is 4 tensor_mul ops ga[32j:32j+32,:] = v[:, j:...] * invw[:, j:...]
    (gb would be ga shifted 4 columns, so only ga[128, TS+4] is materialized
    and the STFT matmuls read ga[:,0:TS] and ga[:,4:4+TS]).
  * STFT: 4 K=128 fp32 matmuls -> t2r (Re f=0..127), t2i (row0 = Re Nyquist,
    rows 1..127 = Im f=1..127).
  * Norm z/|z| without materializing phase: w2 = t2r^2+t2i^2, |z| = Sqrt
    (ACT, the only ACT op), 1/|z| via reciprocal_approx_fast (single DVE op),
    sa/sb rows 1..127 = mag * t2 * inv; rows 0 (DC/Nyquist, where Im==0) are
    sign-patched with mag*sign(Re), scheduled on DVE during the ACT Sqrt.
"""

import numpy as np
from contextlib import ExitStack

import concourse.bass as bass
import concourse.tile as tile
from concourse import bacc, mybir
from concourse import bass_utils

F32 = mybir.dt.float32
I32 = mybir.dt.int32
AF = mybir.ActivationFunctionType
OP = mybir.AluOpType

TC = 64           # cropped frame count (of 1000)
TS = TC - 7       # stft / phase-update frame count
PAD = 7
LP = 3            # left zero-pad of the fused ISTFT psum tile
EPSNY = 2.0 ** -20   # Nyquist basis scale (keeps sign info, kills norm term)
N_ITER = 32
N_FFT = 256
NF = 129
HOP = 32
N_CORES = 8
B = 4


def _consts():
    n = np.arange(N_FFT, dtype=np.float64)
    win = 0.5 - 0.5 * np.cos(2.0 * np.pi * n / N_FFT)
    k = np.arange(128, dtype=np.float64)[:, None]
    ang = 2.0 * np.pi * k * n[None, :] / N_FFT
    ck = np.where(k == 0, 1.0, 2.0) / N_FFT
    a_r = (ck * np.cos(ang) * win[None, :]).astype(np.float32)       # (128,256)
    a_i = (-2.0 / N_FFT * np.sin(ang) * win[None, :]).astype(np.float32)
    a_i[0] = (np.cos(np.pi * n) / N_FFT * win).astype(np.float32)    # Nyquist row

    f = np.arange(128, dtype=np.float64)[None, :]
    ang2 = 2.0 * np.pi * f * n[:, None] / N_FFT                      # (256,128)
    bc = (win[:, None] * np.cos(ang2)).astype(np.float32)
    bi = (-win[:, None] * np.sin(ang2)).astype(np.float32)
    bi[:, 0] = (win * np.cos(np.pi * n)).astype(np.float32)

    L = TC * HOP
    wsq = np.zeros((TC + 8) * HOP + N_FFT, dtype=np.float64)
    w2 = win ** 2
    for t in range(TC + 8):
        s = t * HOP
        wsq[s:s + N_FFT] += w2
    wsq = np.maximum(wsq[:L], 1e-8)
    invwsq = (1.0 / wsq).astype(np.float32).reshape(TC, HOP).T.copy()  # (32, TC)
    return a_r, a_i, bc.copy(), bi.copy(), invwsq


def _emit(tc_ctx, aps, rep=1):
    tc = tc_ctx
    nc = tc.nc
    with ExitStack() as ctx:
        consts = ctx.enter_context(tc.tile_pool(name="consts", bufs=1))
        state = ctx.enter_context(tc.tile_pool(name="state", bufs=1))
        work = ctx.enter_context(tc.tile_pool(name="work", bufs=2))
        psum = ctx.enter_context(tc.tile_pool(name="psum", bufs=1, space="PSUM"))

        a_r = consts.tile([128, 256], F32)
        a_i = consts.tile([128, 256], F32)
        bca = consts.tile([128, 128], F32)
        bcb = consts.tile([128, 128], F32)
        bia = consts.tile([128, 128], F32)
        bib = consts.tile([128, 128], F32)
        invw = consts.tile([32, TC], F32)
        maga = consts.tile([128, TS], F32)
        magn = consts.tile([1, TS], F32)
        W = TC + 2 * PAD
        sab = state.tile([128, 2 * W], F32)   # [sa | sb], one DMA per rep
        sa = sab[:, 0:W]
        sb = sab[:, W:2 * W]

        for t, name in [(a_r, "a_r"), (a_i, "a_i"), (bca, "bca"), (bcb, "bcb"),
                        (bia, "bia"), (bib, "bib"),
                        (invw, "invw"), (maga, "maga"), (magn, "magn")]:
            nc.sync.dma_start(out=t, in_=aps[name])

        if rep > 1:
            from concourse.engine_type import EngineType
            loop = tc.For_i(0, rep, 1, hint_engines=(
                EngineType.PE, EngineType.DVE, EngineType.Activation,
                EngineType.SP))
        else:
            loop = None
        if loop is not None:
            loop.__enter__()
        nc.sync.dma_start(out=sab, in_=aps["sab0"])

        for it in range(N_ITER):
            last = it == N_ITER - 1
            # ---- fused ISTFT (+pair overlap-add): P[32g+i, LP+c] =
            #      p1[32g+i, c] + p2[32g+i, c-4], groups g=0..3. sa-fed
            #      matmuls first: they only wait on the sa write, while the
            #      sb write + Nyquist patch finish in parallel. ----
            P = psum.tile([128, LP + TC], F32, tag="P")
            nc.tensor.matmul(P[:, 0:LP + TC], a_r[:, 0:128],
                             sa[:, PAD - LP:PAD + TC], start=True, stop=False)
            nc.tensor.matmul(P[:, LP + 4:LP + TC], a_r[:, 128:256],
                             sa[:, PAD:PAD + TC - 4], start=False, stop=False)
            nc.tensor.matmul(P[:, 0:LP + TC], a_i[:, 0:128],
                             sb[:, PAD - LP:PAD + TC], start=False, stop=False)
            nc.tensor.matmul(P[:, LP + 4:LP + TC], a_i[:, 128:256],
                             sb[:, PAD:PAD + TC - 4], start=False, stop=True)
            # ---- overlap-add: at most ONE PSUM operand per elementwise op,
            #      so fold the 4 partition groups with an in-place chain ----
            v = work.tile([32, TC], F32, tag="v")
            nc.vector.tensor_copy(v, P[0:32, LP:LP + TC])
            nc.vector.tensor_add(v, v, P[32:64, LP - 1:LP + TC - 1])
            nc.vector.tensor_add(v, v, P[64:96, LP - 2:LP + TC - 2])
            nc.vector.tensor_add(v, v, P[96:128, LP - 3:LP + TC - 3])

            if last:
                wav = work.tile([32, 32], F32, tag="wav")
                nc.vector.tensor_mul(wav, v[:, 0:32], invw[:, 0:32])
                nc.sync.dma_start(out=aps["out"], in_=wav)
                break

            # ---- frame gather with fused 1/win_sq scaling ----
            ga = work.tile([128, TS + 4], F32, tag="ga")
            for j in range(4):
                nc.vector.tensor_mul(ga[32 * j:32 * j + 32, :],
                                     v[:, j:j + TS + 4], invw[:, j:j + TS + 4])
            # ---- STFT: 4 K=128 matmuls on two banks (gb == ga shifted 4
            #      columns). bia/bib column 0 (Nyquist basis) is scaled by
            #      EPSNY, so t2i row 0 ~ 0 and the generic norm stays exact
            #      for row 0 of sa; the Nyquist sign survives in t2i[0]. ----
            t2r = psum.tile([128, TS], F32, tag="t2r")
            t2i = psum.tile([128, TS], F32, tag="t2i")
            nc.tensor.matmul(t2r, bca, ga[:, 0:TS], start=True, stop=False)
            nc.tensor.matmul(t2i, bia, ga[:, 0:TS], start=True, stop=False)
            nc.tensor.matmul(t2r, bcb, ga[:, 4:4 + TS], start=False, stop=True)
            nc.tensor.matmul(t2i, bib, ga[:, 4:4 + TS], start=False, stop=True)

            # ---- phase projection: s' = mag * t2 / |t2| ----
            # squares on ACT (a PSUM-reading 2-tensor mul is illegal; ACT
            # Square has one input, and Square/Sqrt/Copy share one act table)
            u2 = work.tile([128, TS], F32, tag="u2")
            v2 = work.tile([128, TS], F32, tag="v2")
            w2 = work.tile([128, TS], F32, tag="w2")
            nc.scalar.activation(u2, t2r, AF.Square)
            nc.scalar.activation(v2, t2i, AF.Square)
            nc.vector.scalar_tensor_tensor(w2, u2, 1e-12, v2, OP.add, OP.add)
            hyp = work.tile([128, TS], F32, tag="hyp")
            nc.scalar.activation(hyp, w2, AF.Sqrt)
            # Nyquist sign for sb row 0; runs on DVE during the ACT Sqrt
            geB = work.tile([1, TS], F32, tag="geB")
            nc.vector.tensor_scalar(geB, t2i[0:1, :], -1e-6 * EPSNY, 2.0,
                                    OP.is_ge, OP.mult)
            inv = work.tile([128, TS], F32, tag="inv")
            nc.vector.reciprocal_approx_fast(inv, hyp)
            pm = work.tile([128, TS], F32, tag="pm")
            nc.vector.tensor_mul(pm, maga, inv)
            nc.vector.tensor_mul(sa[:, PAD:PAD + TS], t2r, pm)
            nc.vector.tensor_mul(sb[:, PAD:PAD + TS], t2i, pm)
            nc.vector.scalar_tensor_tensor(sb[0:1, PAD:PAD + TS], geB, 1.0,
                                           magn, OP.subtract, OP.mult)
        if loop is not None:
            loop.__exit__(None, None, None)


_CACHED = None


def _build(rep=1):
    global _CACHED
    if rep == 1 and _CACHED is not None:
        return _CACHED
    nc = bacc.Bacc("TRN2", target_bir_lowering=False, debug=False,
                   num_devices=N_CORES)
    shapes = {
        "a_r": (128, 256), "a_i": (128, 256), "bca": (128, 128),
        "bcb": (128, 128), "bia": (128, 128), "bib": (128, 128),
        "invw": (32, TC), "maga": (128, TS),
        "magn": (1, TS),
        "sab0": (128, 2 * (TC + 2 * PAD)),
    }
    aps = {name: nc.dram_tensor(name, shape, F32, kind="ExternalInput").ap()
           for name, shape in shapes.items()}
    aps["out"] = nc.dram_tensor("out", (32, 32), F32, kind="ExternalOutput").ap()
    with tile.TileContext(nc) as t:
        _emit(t, aps, rep=rep)
    nc.compile()
    if rep == 1:
        _CACHED = nc
    return nc


def _host_inputs(mag_b, ph_b):
    """Per-batch host prep: crop, initial cos/sin spec chunks, padding."""
    a_r, a_i, bc, bi, invwsq = _consts()
    mag = np.ascontiguousarray(mag_b[:, :TC]).astype(np.float32)
    ph = np.ascontiguousarray(ph_b[:, :TC]).astype(np.float32)
    W = TC + 2 * PAD
    sab0 = np.zeros((128, 2 * W), np.float32)
    sa0 = sab0[:, 0:W]
    sb0 = sab0[:, W:2 * W]
    sa0[:, PAD:PAD + TC] = mag[0:128] * np.cos(ph[0:128])
    sb0[0, PAD:PAD + TC] = mag[128] * np.cos(ph[128])
    sb0[1:, PAD:PAD + TC] = mag[1:128] * np.sin(ph[1:128])
    bia = np.ascontiguousarray(bi[0:128])
    bib = np.ascontiguousarray(bi[128:256])
    bia[:, 0] *= EPSNY   # t2i row 0 ~ 0 in the norm, keeps the Nyquist sign
    bib[:, 0] *= EPSNY
    return {
        "a_r": a_r, "a_i": a_i,
        "bca": np.ascontiguousarray(bc[0:128]), "bcb": np.ascontiguousarray(bc[128:256]),
        "bia": bia, "bib": bib,
        "invw": invwsq,
        "maga": np.ascontiguousarray(mag[0:128, :TS]),
        "magn": np.ascontiguousarray(mag[128:129, :TS]),
        "sab0": sab0,
    }


def kernel(mag_spec, phase):
    mag_spec = np.asarray(mag_spec, dtype=np.float32)
    phase = np.asarray(phase, dtype=np.float32)
    nc = _build()
    in_maps = [_host_inputs(mag_spec[c % B], phase[c % B]) for c in range(N_CORES)]
    res = bass_utils.run_bass_kernel_spmd(nc, in_maps, core_ids=list(range(N_CORES)))
    out = np.zeros((B, 1000), np.float32)
    for b in range(B):
        blk = res.results[b]["out"]              # (32, 32): [i, m] = wav[32m+i]
        out[b] = blk.T.reshape(-1)[15:1015]
    return out


# revision 43
# speedup vs baseline: 1.9668x; 1.1089x over previous
"""Griffin-Lim phase reconstruction on Trainium2 (Bass/Tile) — v2.

Structure (per core; core c handles batch element c%4, cores 4-7 duplicate):
  * Crop: only the first TC=64 of 1000 STFT frames can affect the returned
    1000 samples (validated numerically: rel err ~6e-5; see mock_numerics.py).
  * Phase carried as (mag*cos, mag*sin); no trig in the loop.
  * ISTFT: 4 K=128 fp32 matmuls accumulate BOTH 128-sample halves into one
    PSUM tile P[128, 3+TC] with the second half at a +4 column offset, so the
    8-way overlap-add collapses to a 2-level tree:
      u[64,1+TC] (PSUM)  = P[0:64] + P[64:128] shifted 2   (1 DVE op)
      v[32,TC]   (SBUF)  = u[0:32] + u[32:64] shifted 1    (1 DVE op)
  * The 1/win_sq normalization is fused into the STFT frame gather: the
    gather is 4 tensor_mul ops ga[32j:32j+32,:] = v[:, j:...] * invw[:, j:...]
    (gb would be ga shifted 4 columns, so only ga[128, TS+4] is materialized
    and the STFT matmuls read ga[:,0:TS] and ga[:,4:4+TS]).
  * STFT: 4 K=128 fp32 matmuls -> t2r (Re f=0..127), t2i (row0 = Re Nyquist,
    rows 1..127 = Im f=1..127).
  * Norm z/|z| without materializing phase: w2 = t2r^2+t2i^2, |z| = Sqrt
    (ACT, the only ACT op), 1/|z| via reciprocal_approx_fast (single DVE op),
    sa/sb rows 1..127 = mag * t2 * inv; rows 0 (DC/Nyquist, where Im==0) are
    sign-patched with mag*sign(Re), scheduled on DVE during the ACT Sqrt.
"""

import numpy as np
from contextlib import ExitStack

import concourse.bass as bass
import concourse.tile as tile
from concourse import bacc, mybir
from concourse import bass_utils

F32 = mybir.dt.float32
I32 = mybir.dt.int32
AF = mybir.ActivationFunctionType
OP = mybir.AluOpType

TC = 56           # cropped frame count (of 1000)
TS = TC - 7       # stft / phase-update frame count
PAD = 7
LP = 3            # left zero-pad of the fused ISTFT psum tile
EPSNY = 2.0 ** -20   # Nyquist basis scale (keeps sign info, kills norm term)
N_ITER = 32
N_FFT = 256
NF = 129
HOP = 32
N_CORES = 8
B = 4


def _consts():
    n = np.arange(N_FFT, dtype=np.float64)
    win = 0.5 - 0.5 * np.cos(2.0 * np.pi * n / N_FFT)
    k = np.arange(128, dtype=np.float64)[:, None]
    ang = 2.0 * np.pi * k * n[None, :] / N_FFT
    ck = np.where(k == 0, 1.0, 2.0) / N_FFT
    a_r = (ck * np.cos(ang) * win[None, :]).astype(np.float32)       # (128,256)
    a_i = (-2.0 / N_FFT * np.sin(ang) * win[None, :]).astype(np.float32)
    a_i[0] = (np.cos(np.pi * n) / N_FFT * win).astype(np.float32)    # Nyquist row

    f = np.arange(128, dtype=np.float64)[None, :]
    ang2 = 2.0 * np.pi * f * n[:, None] / N_FFT                      # (256,128)
    bc = (win[:, None] * np.cos(ang2)).astype(np.float32)
    bi = (-win[:, None] * np.sin(ang2)).astype(np.float32)
    bi[:, 0] = (win * np.cos(np.pi * n)).astype(np.float32)

    L = TC * HOP
    wsq = np.zeros((TC + 8) * HOP + N_FFT, dtype=np.float64)
    w2 = win ** 2
    for t in range(TC + 8):
        s = t * HOP
        wsq[s:s + N_FFT] += w2
    wsq = np.maximum(wsq[:L], 1e-8)
    invwsq = (1.0 / wsq).astype(np.float32).reshape(TC, HOP).T.copy()  # (32, TC)
    return a_r, a_i, bc.copy(), bi.copy(), invwsq


def _emit(tc_ctx, aps, rep=1):
    tc = tc_ctx
    nc = tc.nc
    with ExitStack() as ctx:
        consts = ctx.enter_context(tc.tile_pool(name="consts", bufs=1))
        state = ctx.enter_context(tc.tile_pool(name="state", bufs=1))
        work = ctx.enter_context(tc.tile_pool(name="work", bufs=2))
        psum = ctx.enter_context(tc.tile_pool(name="psum", bufs=1, space="PSUM"))

        a_r = consts.tile([128, 256], F32)
        a_i = consts.tile([128, 256], F32)
        bca = consts.tile([128, 128], F32)
        bcb = consts.tile([128, 128], F32)
        bia = consts.tile([128, 128], F32)
        bib = consts.tile([128, 128], F32)
        invw = consts.tile([32, TC], F32)
        maga = consts.tile([128, TS], F32)
        magn = consts.tile([1, TS], F32)
        W = TC + 2 * PAD
        sab = state.tile([128, 2 * W], F32)   # [sa | sb], one DMA per rep
        sa = sab[:, 0:W]
        sb = sab[:, W:2 * W]

        for t, name in [(a_r, "a_r"), (a_i, "a_i"), (bca, "bca"), (bcb, "bcb"),
                        (bia, "bia"), (bib, "bib"),
                        (invw, "invw"), (maga, "maga"), (magn, "magn")]:
            nc.sync.dma_start(out=t, in_=aps[name])

        if rep > 1:
            from concourse.engine_type import EngineType
            loop = tc.For_i(0, rep, 1, hint_engines=(
                EngineType.PE, EngineType.DVE, EngineType.Activation,
                EngineType.SP))
        else:
            loop = None
        if loop is not None:
            loop.__enter__()
        nc.sync.dma_start(out=sab, in_=aps["sab0"])

        for it in range(N_ITER):
            last = it == N_ITER - 1
            # ---- fused ISTFT (+pair overlap-add): P[32g+i, LP+c] =
            #      p1[32g+i, c] + p2[32g+i, c-4], groups g=0..3. sa-fed
            #      matmuls first: they only wait on the sa write, while the
            #      sb write + Nyquist patch finish in parallel. ----
            P = psum.tile([128, LP + TC], F32, tag="P")
            nc.tensor.matmul(P[:, 0:LP + TC], a_r[:, 0:128],
                             sa[:, PAD - LP:PAD + TC], start=True, stop=False)
            nc.tensor.matmul(P[:, LP + 4:LP + TC], a_r[:, 128:256],
                             sa[:, PAD:PAD + TC - 4], start=False, stop=False)
            nc.tensor.matmul(P[:, 0:LP + TC], a_i[:, 0:128],
                             sb[:, PAD - LP:PAD + TC], start=False, stop=False)
            nc.tensor.matmul(P[:, LP + 4:LP + TC], a_i[:, 128:256],
                             sb[:, PAD:PAD + TC - 4], start=False, stop=True)
            # ---- overlap-add: at most ONE PSUM operand per elementwise op,
            #      so fold the 4 partition groups with an in-place chain ----
            v = work.tile([32, TC], F32, tag="v")
            nc.vector.tensor_copy(v, P[0:32, LP:LP + TC])
            nc.vector.tensor_add(v, v, P[32:64, LP - 1:LP + TC - 1])
            nc.vector.tensor_add(v, v, P[64:96, LP - 2:LP + TC - 2])
            nc.vector.tensor_add(v, v, P[96:128, LP - 3:LP + TC - 3])

            if last:
                wav = work.tile([32, 32], F32, tag="wav")
                nc.vector.tensor_mul(wav, v[:, 0:32], invw[:, 0:32])
                nc.sync.dma_start(out=aps["out"], in_=wav)
                break

            # ---- frame gather with fused 1/win_sq scaling ----
            ga = work.tile([128, TS + 4], F32, tag="ga")
            for j in range(4):
                nc.vector.tensor_mul(ga[32 * j:32 * j + 32, :],
                                     v[:, j:j + TS + 4], invw[:, j:j + TS + 4])
            # ---- STFT: 4 K=128 matmuls on two banks (gb == ga shifted 4
            #      columns). bia/bib column 0 (Nyquist basis) is scaled by
            #      EPSNY, so t2i row 0 ~ 0 and the generic norm stays exact
            #      for row 0 of sa; the Nyquist sign survives in t2i[0]. ----
            t2r = psum.tile([128, TS], F32, tag="t2r")
            t2i = psum.tile([128, TS], F32, tag="t2i")
            nc.tensor.matmul(t2r, bca, ga[:, 0:TS], start=True, stop=False)
            nc.tensor.matmul(t2i, bia, ga[:, 0:TS], start=True, stop=False)
            nc.tensor.matmul(t2r, bcb, ga[:, 4:4 + TS], start=False, stop=True)
            nc.tensor.matmul(t2i, bib, ga[:, 4:4 + TS], start=False, stop=True)

            # ---- phase projection: s' = mag * t2 / |t2| ----
            # squares on ACT (a PSUM-reading 2-tensor mul is illegal; ACT
            # Square has one input, and Square/Sqrt/Copy share one act table)
            u2 = work.tile([128, TS], F32, tag="u2")
            v2 = work.tile([128, TS], F32, tag="v2")
            w2 = work.tile([128, TS], F32, tag="w2")
            nc.scalar.activation(u2, t2r, AF.Square)
            nc.scalar.activation(v2, t2i, AF.Square)
            nc.vector.scalar_tensor_tensor(w2, u2, 1e-12, v2, OP.add, OP.add)
            hyp = work.tile([128, TS], F32, tag="hyp")
            nc.scalar.activation(hyp, w2, AF.Sqrt)
            # Nyquist sign for sb row 0; runs on DVE during the ACT Sqrt
            geB = work.tile([1, TS], F32, tag="geB")
            nc.vector.tensor_scalar(geB, t2i[0:1, :], -1e-6 * EPSNY, 2.0,
                                    OP.is_ge, OP.mult)
            inv = work.tile([128, TS], F32, tag="inv")
            nc.vector.reciprocal_approx_fast(inv, hyp)
            pm = work.tile([128, TS], F32, tag="pm")
            nc.vector.tensor_mul(pm, maga, inv)
            nc.vector.tensor_mul(sa[:, PAD:PAD + TS], t2r, pm)
            nc.vector.tensor_mul(sb[:, PAD:PAD + TS], t2i, pm)
            nc.vector.scalar_tensor_tensor(sb[0:1, PAD:PAD + TS], geB, 1.0,
                                           magn, OP.subtract, OP.mult)
        if loop is not None:
            loop.__exit__(None, None, None)


_CACHED = None


def _build(rep=1):
    global _CACHED
    if rep == 1 and _CACHED is not None:
        return _CACHED
    nc = bacc.Bacc("TRN2", target_bir_lowering=False, debug=False,
                   num_devices=N_CORES)
    shapes = {
        "a_r": (128, 256), "a_i": (128, 256), "bca": (128, 128),
        "bcb": (128, 128), "bia": (128, 128), "bib": (128, 128),
        "invw": (32, TC), "maga": (128, TS),
        "magn": (1, TS),
        "sab0": (128, 2 * (TC + 2 * PAD)),
    }
    aps = {name: nc.dram_tensor(name, shape, F32, kind="ExternalInput").ap()
           for name, shape in shapes.items()}
    aps["out"] = nc.dram_tensor("out", (32, 32), F32, kind="ExternalOutput").ap()
    with tile.TileContext(nc) as t:
        _emit(t, aps, rep=rep)
    nc.compile()
    if rep == 1:
        _CACHED = nc
    return nc


def _host_inputs(mag_b, ph_b):
    """Per-batch host prep: crop, initial cos/sin spec chunks, padding."""
    a_r, a_i, bc, bi, invwsq = _consts()
    mag = np.ascontiguousarray(mag_b[:, :TC]).astype(np.float32)
    ph = np.ascontiguousarray(ph_b[:, :TC]).astype(np.float32)
    W = TC + 2 * PAD
    sab0 = np.zeros((128, 2 * W), np.float32)
    sa0 = sab0[:, 0:W]
    sb0 = sab0[:, W:2 * W]
    sa0[:, PAD:PAD + TC] = mag[0:128] * np.cos(ph[0:128])
    sb0[0, PAD:PAD + TC] = mag[128] * np.cos(ph[128])
    sb0[1:, PAD:PAD + TC] = mag[1:128] * np.sin(ph[1:128])
    bia = np.ascontiguousarray(bi[0:128])
    bib = np.ascontiguousarray(bi[128:256])
    bia[:, 0] *= EPSNY   # t2i row 0 ~ 0 in the norm, keeps the Nyquist sign
    bib[:, 0] *= EPSNY
    return {
        "a_r": a_r, "a_i": a_i,
        "bca": np.ascontiguousarray(bc[0:128]), "bcb": np.ascontiguousarray(bc[128:256]),
        "bia": bia, "bib": bib,
        "invw": invwsq,
        "maga": np.ascontiguousarray(mag[0:128, :TS]),
        "magn": np.ascontiguousarray(mag[128:129, :TS]),
        "sab0": sab0,
    }


def kernel(mag_spec, phase):
    mag_spec = np.asarray(mag_spec, dtype=np.float32)
    phase = np.asarray(phase, dtype=np.float32)
    nc = _build()
    in_maps = [_host_inputs(mag_spec[c % B], phase[c % B]) for c in range(N_CORES)]
    res = bass_utils.run_bass_kernel_spmd(nc, in_maps, core_ids=list(range(N_CORES)))
    out = np.zeros((B, 1000), np.float32)
    for b in range(B):
        blk = res.results[b]["out"]              # (32, 32): [i, m] = wav[32m+i]
        out[b] = blk.T.reshape(-1)[15:1015]
    return out
